# revision 14
# baseline (speedup 1.0000x reference)
"""Single-head attention (SEQ=8192, D_MODEL=2048, D_K=128) on 8 TRN2 NeuronCores.

Sharding: sequence-parallel. Each core owns 1024 query rows. QKV projections
computed on the local shard; K^T and V all-gathered across the 8 cores
(d_k=128 lives on the partition dim, so gathered blocks DMA straight into
matmul operand layouts with no device-side transposes). Attention runs in
S^T layout ([key, query] tiles): exp on the scalar engine, P@V and the
softmax denominator both accumulated on the tensor engine, with the PE/ACT
dependency software-pipelined one step apart.
"""
import os

import numpy as np

import concourse.bacc as bacc
import concourse.tile as tile
import concourse.masks as masks
from concourse import mybir
from concourse.bass_utils import run_bass_kernel_spmd

N_CORES = 8
SEQ = 8192
DM = 2048
DK = 128
SL = SEQ // N_CORES          # 1024 local rows
NMC = DM // 128              # 16 contraction chunks for projections
NKB = SEQ // 128             # 64 key blocks
NQS = SL // 512              # 2 query slabs of 512
SCALE = float(np.sqrt(DK))

F32 = mybir.dt.float32


def _build(mm_dt, cut="full"):
    nc = bacc.Bacc(
        "TRN2",
        target_bir_lowering=False,
        debug=False,
        num_devices=N_CORES,
    )

    xT = nc.dram_tensor("xT", [DM, SL], mm_dt, kind="ExternalInput")
    wqT = nc.dram_tensor("wqT", [DM, DK], mm_dt, kind="ExternalInput")
    wkT = nc.dram_tensor("wkT", [DM, DK], mm_dt, kind="ExternalInput")
    wvT = nc.dram_tensor("wvT", [DM, DK], mm_dt, kind="ExternalInput")
    bq = nc.dram_tensor("bq", [DK, 1], F32, kind="ExternalInput")
    bk = nc.dram_tensor("bk", [DK, 1], F32, kind="ExternalInput")
    bv = nc.dram_tensor("bv", [DK, 1], F32, kind="ExternalInput")
    ones_d = nc.dram_tensor("ones_d", [128, 128], mm_dt, kind="ExternalInput")
    ident_d = nc.dram_tensor("ident_d", [128, 128], mm_dt, kind="ExternalInput")
    identf_d = nc.dram_tensor("identf_d", [128, 128], F32, kind="ExternalInput")
    out = nc.dram_tensor("out", [SL, DK], F32, kind="ExternalOutput")

    CP = mybir.ActivationFunctionType.Copy
    ID = mybir.ActivationFunctionType.Identity

    with tile.TileContext(nc) as tc:
        with (
            tc.tile_pool(name="const", bufs=1) as const_pool,
            tc.tile_pool(name="w", bufs=1) as w_pool,
            tc.tile_pool(name="xc", bufs=4) as x_pool,
            tc.tile_pool(name="proj", bufs=1) as proj_pool,
            tc.tile_pool(name="kv", bufs=1) as kv_pool,
            tc.tile_pool(name="pt", bufs=3) as pt_pool,
            tc.tile_pool(name="fin", bufs=2) as fin_pool,
            tc.tile_pool(name="dram", bufs=1, space="DRAM") as dram_pool,
        ):
            # ---- constants ----
            ident = const_pool.tile([128, 128], F32)
            nc.sync.dma_start(ident[:], identf_d[:])
            ident_r = const_pool.tile([128, 128], mm_dt)
            nc.sync.dma_start(ident_r[:], ident_d[:])
            ones = const_pool.tile([128, 128], mm_dt)
            nc.sync.dma_start(ones[:], ones_d[:])
            bq_sb = const_pool.tile([DK, 1], F32)
            bk_sb = const_pool.tile([DK, 1], F32)
            bv_sb = const_pool.tile([DK, 1], F32)
            nc.sync.dma_start(bq_sb[:], bq[:])
            nc.sync.dma_start(bk_sb[:], bk[:])
            nc.sync.dma_start(bv_sb[:], bv[:])

            # ---- weights: [128, 16, 128], chunk i = W.T rows i*128..i*128+127 ----
            wq_sb = w_pool.tile([128, NMC, DK], mm_dt)
            wk_sb = w_pool.tile([128, NMC, DK], mm_dt)
            wv_sb = w_pool.tile([128, NMC, DK], mm_dt)
            nc.sync.dma_start(wq_sb[:], wqT.rearrange("(c p) d -> p c d", p=128))
            nc.sync.dma_start(wk_sb[:], wkT.rearrange("(c p) d -> p c d", p=128))
            nc.sync.dma_start(wv_sb[:], wvT.rearrange("(c p) d -> p c d", p=128))

            qt_sb = proj_pool.tile([128, SL], mm_dt)
            kt_loc = proj_pool.tile([128, SL], mm_dt)
            vt_loc = proj_pool.tile([128, SL], mm_dt)
            kt_dram = dram_pool.tile([128, SL], mm_dt)
            vc_dram = dram_pool.tile([SL, DK], mm_dt)

            # ---- phase A: projections QT/KT/VT [128, 1024] = W @ x_loc^T (+bias)
            with tc.tile_pool(name="ps_proj", bufs=1, space="PSUM") as ps_proj:
                qt_ps = ps_proj.tile([128, SL], F32)
                kt_ps = ps_proj.tile([128, SL], F32)
                vt_ps = ps_proj.tile([128, SL], F32)
                for i in range(NMC):
                    xc = x_pool.tile([128, SL], mm_dt)
                    nc.sync.dma_start(xc[:], xT[i * 128:(i + 1) * 128, :])
                    for h in range(2):
                        hs = slice(h * 512, (h + 1) * 512)
                        nc.tensor.matmul(qt_ps[:, hs], wq_sb[:, i, :], xc[:, hs],
                                         start=(i == 0), stop=(i == NMC - 1))
                        nc.tensor.matmul(kt_ps[:, hs], wk_sb[:, i, :], xc[:, hs],
                                         start=(i == 0), stop=(i == NMC - 1))
                        nc.tensor.matmul(vt_ps[:, hs], wv_sb[:, i, :], xc[:, hs],
                                         start=(i == 0), stop=(i == NMC - 1))

                nc.scalar.activation(qt_sb[:], qt_ps[:], ID, bias=bq_sb[:])
                nc.scalar.activation(kt_loc[:], kt_ps[:], ID, bias=bk_sb[:])
                nc.scalar.activation(vt_loc[:], vt_ps[:], ID, bias=bv_sb[:])

                # bounce local K^T to DRAM for the all-gather
                nc.sync.dma_start(kt_dram[:], kt_loc[:])
                # V natural = transpose of VT blocks (PE transpose)
                with tc.tile_pool(name="ps_vtr", bufs=2, space="PSUM") as ps_vtr:
                    for t in range(SL // 128):
                        vtr = ps_vtr.tile([128, 128], mm_dt, tag="vtr")
                        nc.tensor.transpose(
                            vtr[:], vt_loc[:, t * 128:(t + 1) * 128], ident_r[:])
                        vsb = x_pool.tile([128, 128], mm_dt, tag="vsb")
                        nc.scalar.copy(vsb[:], vtr[:])
                        nc.sync.dma_start(vc_dram[t * 128:(t + 1) * 128, :], vsb[:])

            if cut == "proj":
                nc.sync.dma_start(out[0:128, :], qt_sb[:, 0:128].bitcast(F32))
                nc.compile_hook_done = True
            ktg_dram = dram_pool.tile([N_CORES, 128, SL], mm_dt, addr_space="Shared")
            vg_dram = dram_pool.tile([N_CORES, SL, DK], mm_dt, addr_space="Shared")
            groups = [list(range(N_CORES))]
            if cut == "proj":
                groups = None  # sentinel; sections below are skipped
            if groups: nc.gpsimd.collective_compute(
                "AllGather", mybir.AluOpType.bypass, replica_groups=groups,
                ins=[kt_dram.opt()], outs=[ktg_dram.opt()],
            )
            if groups: nc.gpsimd.collective_compute(
                "AllGather", mybir.AluOpType.bypass, replica_groups=groups,
                ins=[vc_dram.opt()], outs=[vg_dram.opt()],
            )

            # ---- stage gathered K^T [128, 8192] and V blocks [128, 64, 128] ----
            kt_sb = kv_pool.tile([128, SEQ], mm_dt)
            v_sb = kv_pool.tile([128, NKB, DK], mm_dt)
            for b in range(N_CORES if groups else 0):
                nc.sync.dma_start(kt_sb[:, b * SL:(b + 1) * SL], ktg_dram[b])
                nc.sync.dma_start(
                    v_sb[:, b * (SL // 128):(b + 1) * (SL // 128), :],
                    vg_dram[b].rearrange("(t p) d -> p t d", p=128),
                )

            # ---- phase B: attention, S^T layout, software-pipelined ----
            if cut == "gather":
                nc.sync.dma_start(out[0:128, :], v_sb[:, 0, :].bitcast(F32))
            cs_dram = dram_pool.tile([NQS, 512], F32)
            NJJ = NKB // 2  # pairs of key blocks
            if cut != "full":
                NJJ = 0
                NQS_eff = 0
            else:
                NQS_eff = NQS
            with (
                tc.tile_pool(name="ps_st", bufs=2, space="PSUM") as ps_st,
                tc.tile_pool(name="ps_o", bufs=2, space="PSUM") as ps_o,
            ):
                for qs in range(NQS_eff):
                    q_rhs = qt_sb[:, qs * 512:(qs + 1) * 512]
                    o_ps = ps_o.tile([128, 512], F32, tag="o")
                    cs_ps = (None if cut == "nocs"
                             else ps_o.tile([128, 512], F32, tag="cs"))
                    pts = {}
                    for jj in range(NJJ + 1):
                        if jj < NJJ:
                            st_ps = ps_st.tile([128, 1024], F32, tag="st")
                            for u in range(2):
                                j = 2 * jj + u
                                nc.tensor.matmul(
                                    st_ps[:, u * 512:(u + 1) * 512],
                                    kt_sb[:, j * 128:(j + 1) * 128], q_rhs,
                                    start=True, stop=True,
                                )
                            pt = pt_pool.tile([128, 1024], mm_dt, tag="pt")
                            nc.scalar.activation(
                                pt[:], st_ps[:], mybir.ActivationFunctionType.Exp)
                            pts[jj] = pt
                        if jj > 0:
                            pt = pts.pop(jj - 1)
                            for u in range(2):
                                j = 2 * (jj - 1) + u
                                pslice = pt[:, u * 512:(u + 1) * 512]
                                nc.tensor.matmul(o_ps[:], v_sb[:, j, :], pslice,
                                                 start=(j == 0), stop=(j == NKB - 1))
                                if cs_ps is not None:
                                    nc.tensor.matmul(
                                        cs_ps[:], ones[:], pslice,
                                        start=(j == 0), stop=(j == NKB - 1))

                    # softmax denominators: reciprocal + normalize O^T directly
                    o_sb = fin_pool.tile([128, 512], F32, tag="o_sb")
                    if cs_ps is not None:
                        rcs = fin_pool.tile([128, 512], F32, tag="rcs")
                        nc.vector.reciprocal(rcs[:], cs_ps[:])
                        nc.vector.tensor_mul(o_sb[:], o_ps[:], rcs[:])
                    else:
                        nc.vector.tensor_copy(o_sb[:], o_ps[:])
                    for t in range(4):
                        otr = ps_st.tile([128, 128], F32, tag="st")
                        nc.tensor.transpose(
                            otr[:], o_sb[:, t * 128:(t + 1) * 128], ident[:])
                        oo = fin_pool.tile([128, DK], F32, tag="oo")
                        nc.scalar.activation(oo[:], otr[:], CP)
                        nc.sync.dma_start(
                            out[qs * 512 + t * 128:qs * 512 + (t + 1) * 128, :],
                            oo[:])

    nc.compile()
    return nc


_NC_CACHE = {}


def _get_nc(mm_dt):
    import os as _os
    cut = _os.environ.get("KCUT", "full")
    key = (str(mm_dt), cut)
    if key not in _NC_CACHE:
        _NC_CACHE[key] = _build(mm_dt, cut)
    return _NC_CACHE[key]


def _run(inputs, trace=False, mm_dt=None, **spmd_kwargs):
    if mm_dt is None:
        mm_dt = (mybir.dt.float32r
                 if os.environ.get("KDT", "f32r") == "f32r" else F32)
    x = np.asarray(inputs["x"], dtype=np.float32)
    Wq = np.asarray(inputs["Wq"], dtype=np.float32)
    Wk = np.asarray(inputs["Wk"], dtype=np.float32)
    Wv = np.asarray(inputs["Wv"], dtype=np.float32)
    bq = np.asarray(inputs["bq"], dtype=np.float32)
    bk = np.asarray(inputs["bk"], dtype=np.float32)
    bv = np.asarray(inputs["bv"], dtype=np.float32)

    wqT = np.ascontiguousarray((Wq / SCALE).T)
    wkT = np.ascontiguousarray(Wk.T)
    wvT = np.ascontiguousarray(Wv.T)
    shared = {
        "wqT": wqT, "wkT": wkT, "wvT": wvT,
        "bq": np.ascontiguousarray((bq / SCALE)[:, None]),
        "bk": np.ascontiguousarray(bk[:, None]),
        "bv": np.ascontiguousarray(bv[:, None]),
        "ones_d": np.ones((128, 128), dtype=np.float32),
        "ident_d": np.eye(128, dtype=np.float32),
        "identf_d": np.eye(128, dtype=np.float32),
    }
    in_maps = []
    for c in range(N_CORES):
        xT_c = np.ascontiguousarray(x[c * SL:(c + 1) * SL].T)
        in_maps.append({"xT": xT_c, **shared})

    nc = _get_nc(mm_dt)
    res = run_bass_kernel_spmd(
        nc, in_maps, core_ids=list(range(N_CORES)), trace=trace, **spmd_kwargs)
    full = np.concatenate([res.results[c]["out"] for c in range(N_CORES)], axis=0)
    return full, res


def kernel(**inputs):
    out, _ = _run(inputs)
    return out
